# revision 40
# baseline (speedup 1.0000x reference)
# Trainium2 Bass kernel for Bahdanau-style attention (nn_Attention).
#
# reference math (per batch b):
#   h_part = hiddens[b] @ Wd[:DH]                # [S, A]
#   feat   = tanh(h_part + pattern[b] @ Wd[DH:] + bd)
#   score  = feat @ Wv + bv                      # [S, 1]
#   w      = softmax(score over S)               # mask is all-ones
#   out[b] = sum_s w[s] * hiddens[b, s]          # [DH]
#
# Strategy: data-parallel over batch across 8 cores (4 batches/core),
# weights replicated.  Scores are tanh-bounded (|score| <~ 25) so the
# softmax is computed unnormalized: acc = sum exp(s)*h, l = sum exp(s),
# out = acc / l -- a single pass over hiddens, nothing big materialized.
#
# The host stages hiddens pre-transposed per core ([DH, S] per batch) so
# the device reads it exactly once, d-major -- the layout both consumers
# want.  Per-core dataflow (bf16 compute, f32 accumulation):
#   - SWDGE DMA loads hiddensT with f32->bf16 cast: hT [128 d, dj, s]
#   - mm1 (PE): psum[a, s] += Wd_bf[dj, a].T @ hT[dj, s] over 8 d-chunks
#   - ACT: feat = tanh(psum + bias[a]), bias = pattern@Wd_p + bd fused
#     as a per-partition scalar in the [a, s] layout
#   - mm-score (PE): psum[1, s] += Wv[a].T @ feat[a, s] over 4 a-chunks
#   - ACT: e = exp(score + bv) -> [1, S] row; accum_out gives sum(e)
#   - weighted sum on the (otherwise idle) Vector engine:
#     ctx[d] = sum_s hT[d, s] * e[s] via affine_mul_reduce against an
#     e row broadcast across partitions by a tiny ones-matmul -- no
#     transposes needed anywhere on the wide data path
#   - out[b] = ctx / l via a tiny 1/l broadcast matmul + scalar multiply

import numpy as np
from contextlib import ExitStack

B, S, DH, P, A = 32, 2048, 1024, 512, 512
NCORES = 8
BPC = B // NCORES          # batches per core
NT = 4                     # s-tiles of 512 per batch
DCH = DH // 128            # 8 d-chunks
ACH = A // 128             # 4 a-chunks
PCH = P // 128             # 4 p-chunks

_graph_cache = {}


def _force_after(later, earlier):
    # scheduler hint: `later` must come after `earlier` in engine order
    from concourse.tile_rust import add_dep_helper
    li = getattr(later, "instruction", None) or getattr(later, "ins", later)
    ei = getattr(earlier, "instruction", None) or getattr(earlier, "ins", earlier)
    add_dep_helper(li, ei, sync=False, reason="keep tail after chain")


def _build_graph():
    import concourse.bass as bass
    import concourse.mybir as mybir
    import concourse.tile as tile
    from concourse import bacc

    F32 = mybir.dt.float32
    BF16 = mybir.dt.bfloat16
    Act = mybir.ActivationFunctionType

    nc = bacc.Bacc("TRN2", target_bir_lowering=False, debug=False,
                   num_devices=NCORES)

    hT_in = nc.dram_tensor("hiddensT", [BPC, DH, S], F32, kind="ExternalInput").ap()
    wd_in = nc.dram_tensor("Wd", [DH + P, A], F32, kind="ExternalInput").ap()
    # cpack[:, 0:4]=bd, [:, 4:8]=Wv, [:, 8:24]=patternT (c-major), [:, 24]=bv
    cp_in = nc.dram_tensor("cpack", [128, 25], F32, kind="ExternalInput").ap()
    out = nc.dram_tensor("out", [BPC, 128, DCH], F32, kind="ExternalOutput").ap()

    with tile.TileContext(nc) as tc:
        with ExitStack() as es:
            _body(es, tc, nc, mybir, F32, BF16, Act,
                  out, hT_in, wd_in, cp_in)
    # run_bass_via_pjrt binds the exec primitive directly and skips the
    # finalize that runs bacc's register-allocation pass -- do it here.
    nc.finalize()
    return nc


def _body(es, tc, nc, mybir, F32, BF16, Act, out, hT_in, wd_in, cp_in):
    const = es.enter_context(tc.tile_pool(name="const", bufs=1))
    hpool = es.enter_context(tc.tile_pool(name="hp", bufs=4))
    fpool = es.enter_context(tc.tile_pool(name="fp", bufs=2))
    epool = es.enter_context(tc.tile_pool(name="ep", bufs=2))
    opool = es.enter_context(tc.tile_pool(name="op", bufs=4))
    ps_mm1 = es.enter_context(tc.tile_pool(name="ps_mm1", bufs=2, space="PSUM"))
    ps_sc = es.enter_context(tc.tile_pool(name="ps_sc", bufs=2, space="PSUM"))
    ps_ebc = es.enter_context(tc.tile_pool(name="ps_ebc", bufs=4, space="PSUM"))

    # ---- constants / weights ----
    # SWDGE queue: Wd a-cols 0:256 first (so mm1 a=0/1 can start), then
    # batch 0's first small s-slice; the packed small constants ride the
    # HWDGE queue in parallel and are cast/sliced on-chip
    wd_bf = const.tile([128, DCH + PCH, A], BF16, tag="wd")
    wd_src = wd_in.rearrange("(c p) a -> p c a", p=128)
    nc.gpsimd.dma_start(wd_bf[:, :, 0:256], wd_src[:, :, 0:256])

    cpack = const.tile([128, 25], F32, tag="cpack")
    nc.sync.dma_start(cpack[:], cp_in[:])
    bd_sb = cpack[:, 0:4]
    bv_sb = cpack[0:1, 24:25]
    wv_bf = const.tile([128, ACH], BF16, tag="wv")
    nc.scalar.activation(wv_bf[:], cpack[:, 4:8], Act.Identity)
    patT_bf = const.tile([128, PCH * BPC], BF16, tag="patT")
    nc.scalar.activation(patT_bf[:], cpack[:, 8:24], Act.Identity)

    hT0 = hpool.tile([128, DCH, S], BF16, tag="h")
    h0src = hT_in[0].rearrange("(j p) s -> p j s", p=128)
    nc.gpsimd.dma_start(hT0[:, :, 0:256], h0src[:, :, 0:256])
    nc.gpsimd.dma_start(wd_bf[:, :, 256:512], wd_src[:, :, 256:512])
    for sl in [slice(256, 512), slice(512, 1024), slice(1024, 1536),
               slice(1536, 2048)]:
        nc.gpsimd.dma_start(hT0[:, :, sl], h0src[:, :, sl])
    # rows of ones for partition-broadcast matmuls (e rows, 1/l)
    ones_f32 = const.tile([1, 128], F32, tag="ones")
    nc.vector.memset(ones_f32[:], 1.0)
    ones_bf = const.tile([1, 128], BF16, tag="onesb")
    nc.vector.memset(ones_bf[:], 1.0)

    # bias_ab[a, achunk, batch] = (pattern[b] @ Wd_p + bd)[a]; emitted
    # after the first tile's mm1 matmuls (see _emit_bias) so the PE
    # stream is not blocked on the second Wd half at startup
    bias_ab = const.tile([128, ACH, BPC], F32, tag="bias")

    def _emit_bias():
        for a in range(ACH):
            ps_pp = ps_sc.tile([128, 512], F32, tag="sc")
            for k in range(PCH):
                nc.tensor.matmul(
                    ps_pp[:, :BPC],
                    wd_bf[:, DCH + k, a * 128:(a + 1) * 128],
                    patT_bf[:, k * BPC:(k + 1) * BPC],
                    start=(k == 0), stop=(k == PCH - 1),
                )
            nc.vector.tensor_scalar_add(bias_ab[:, a, :], ps_pp[:, :BPC],
                                        bd_sb[:, a:a + 1])


    # ---- main loop over batches ----
    l_rcp_all = epool.tile([1, BPC], F32, tag="lrcpall")
    ctx_list = []
    for b in range(BPC):
        # load hT[b] as bf16: [128 d-part, 8 d-chunk, 2048 s], one DMA per
        # 512-s slice so mm1 of tile t starts as soon as slice t lands
        if b == 0:
            hT = hT0
        else:
            hT = hpool.tile([128, DCH, S], BF16, tag="h")
            hsrc = hT_in[b].rearrange("(j p) s -> p j s", p=128)
            for t in range(NT):
                sl = slice(t * 512, (t + 1) * 512)
                nc.gpsimd.dma_start(hT[:, :, sl], hsrc[:, :, sl])

        e_row = epool.tile([1, S], BF16, tag="erow")
        l_parts = epool.tile([1, NT], F32, tag="lparts")
        e_ps_t = [None] * NT

        for t in range(NT):
            sl = slice(t * 512, (t + 1) * 512)
            # mm1 + tanh -> feat [a-part, achunk, s]
            feat = fpool.tile([128, ACH, 512], BF16, tag="feat")
            first = (b == 0 and t == 0)
            ps1s = [None] * ACH
            for a in range(ACH):
                ps1 = ps_mm1.tile([128, 512], F32, tag="mm1")
                ps1s[a] = ps1
                for dj in range(DCH):
                    nc.tensor.matmul(
                        ps1[:],
                        wd_bf[:, dj, a * 128:(a + 1) * 128],
                        hT[:, dj, sl],
                        start=(dj == 0), stop=(dj == DCH - 1),
                    )
                if not first:
                    nc.scalar.activation(feat[:, a, :], ps1[:], Act.Tanh,
                                         bias=bias_ab[:, a, b:b + 1])
            if first:
                _emit_bias()
                for a in range(ACH):
                    nc.scalar.activation(feat[:, a, :], ps1s[a][:], Act.Tanh,
                                         bias=bias_ab[:, a, b:b + 1])

            # score [1, 512]
            ps_s = ps_sc.tile([1, 512], F32, tag="sc")
            for a in range(ACH):
                nc.tensor.matmul(
                    ps_s[:],
                    wv_bf[:, a:a + 1],
                    feat[:, a, :],
                    start=(a == 0), stop=(a == ACH - 1),
                )

            # e = exp(score + bv) into the batch row; l_t = sum(e)
            nc.scalar.activation(e_row[:, sl], ps_s[:], Act.Exp,
                                 bias=bv_sb[:],
                                 accum_out=l_parts[:, t:t + 1])
            # broadcast e across partitions: ones^T @ e_row -> psum
            e_ps_tile = ps_ebc.tile([128, 512], F32, tag="ebc")
            e_ps_t[t] = e_ps_tile
            nc.tensor.matmul(e_ps_t[t][:], ones_bf[:], e_row[:, sl],
                             start=True, stop=True)

        # weighted sum on DVE: ctx[d-part, dj] = sum_s hT[d, dj, s] * e[s]
        # (in1 streams the broadcast e straight from PSUM); chunked so the
        # chain starts before the last exp -- finer on the last batch to
        # shorten the kernel tail
        nch = NT
        csz = S // nch
        ctx_h = opool.tile([128, DCH, NT], F32, tag="ctxh")
        scratch = fpool.tile([128, S // 2], BF16, tag="scratch")
        first_amr_last = None
        for half in range(nch):
            hs = slice(half * csz, (half + 1) * csz)
            for dj in range(DCH):
                amr = nc.vector.affine_mul_reduce(
                    out=scratch[:, :csz],
                    accum_out=ctx_h[:, dj, half:half + 1],
                    in0=hT[:, dj, hs],
                    in1=e_ps_t[half][:],
                    scale=1.0,
                    bias=0.0,
                )
                if half == nch - 1 and dj == 0:
                    first_amr_last = amr
        ctx_sb = opool.tile([128, DCH], F32, tag="ctx")
        nc.vector.tensor_add(ctx_h[:, :, 0], ctx_h[:, :, 0], ctx_h[:, :, 1])
        nc.vector.tensor_add(ctx_h[:, :, 2], ctx_h[:, :, 2], ctx_h[:, :, 3])
        add3 = nc.vector.tensor_add(ctx_sb[:], ctx_h[:, :, 0], ctx_h[:, :, 2])

        # l sum + reciprocal; ordering edge keeps these late-waiting DVE
        # ops BEHIND the weighted-sum chain in the in-order DVE stream
        l_sum = epool.tile([1, 1], F32, tag="lsum")
        ladd = nc.vector.reduce_sum(l_sum[:], l_parts[:],
                                    axis=mybir.AxisListType.X)
        _force_after(ladd, first_amr_last)
        nc.vector.reciprocal(l_rcp_all[:, b:b + 1], l_sum[:])
        ctx_list.append(ctx_sb)

    # ---- division tail: one broadcast matmul, then scale + store ----
    ps_l = ps_sc.tile([128, 512], F32, tag="sc")
    nc.tensor.matmul(ps_l[:, :BPC], ones_f32[:], l_rcp_all[:],
                     start=True, stop=True)
    for b in range(BPC):
        out_sb = opool.tile([128, DCH], F32, tag="osb")
        nc.vector.tensor_scalar_mul(out_sb[:], ctx_list[b][:], ps_l[:, b:b + 1])
        nc.sync.dma_start(out[b], out_sb[:])


def _get_graph():
    if "nc" not in _graph_cache:
        _graph_cache["nc"] = _build_graph()
    return _graph_cache["nc"]


def _make_in_maps(hiddens, pattern, Wd, bd, Wv, bv):
    hiddens = np.asarray(hiddens, dtype=np.float32)
    pattern = np.asarray(pattern, dtype=np.float32)
    Wd = np.asarray(Wd, dtype=np.float32)
    bd = np.asarray(bd, dtype=np.float32)
    Wv = np.asarray(Wv, dtype=np.float32)
    bv = np.asarray(bv, dtype=np.float32)
    in_maps = []
    for c in range(NCORES):
        sl = slice(c * BPC, (c + 1) * BPC)
        cpack = np.zeros((128, 25), dtype=np.float32)
        cpack[:, 0:4] = np.asarray(bd, np.float32).reshape(ACH, 128).T
        cpack[:, 4:8] = np.asarray(Wv, np.float32).reshape(ACH, 128).T
        # patternT[p, c*BPC + b] = pattern[b, c*128 + p]
        patT = np.asarray(pattern[sl], np.float32).T.reshape(PCH, 128, BPC)
        cpack[:, 8:24] = patT.transpose(1, 0, 2).reshape(128, PCH * BPC)
        cpack[:, 24] = np.float32(np.asarray(bv).reshape(-1)[0])
        in_maps.append({
            "hiddensT": np.ascontiguousarray(
                hiddens[sl].transpose(0, 2, 1), dtype=np.float32),
            "Wd": np.ascontiguousarray(Wd, dtype=np.float32),
            "cpack": cpack,
        })
    return in_maps


def run(hiddens, pattern, mask, Wd, bd, Wv, bv, trace=False, **spmd_kwargs):
    from concourse.bass_utils import run_bass_kernel_spmd
    nc = _get_graph()
    in_maps = _make_in_maps(hiddens, pattern, Wd, bd, Wv, bv)
    res = run_bass_kernel_spmd(nc, in_maps, core_ids=list(range(NCORES)),
                               trace=trace, **spmd_kwargs)
    # device emits [BPC, 128, DCH] with d = dj*128 + p; unpermute here
    outs = [np.asarray(res.results[c]["out"]).transpose(0, 2, 1).reshape(BPC, DH)
            for c in range(NCORES)]
    full = np.concatenate(outs, axis=0).astype(np.float32)
    return full, res


def kernel(hiddens, pattern, mask, Wd, bd, Wv, bv):
    full, _ = run(hiddens, pattern, mask, Wd, bd, Wv, bv, trace=False)
    return full


# revision 41
# speedup vs baseline: 1.0091x; 1.0091x over previous
# Trainium2 Bass kernel for Bahdanau-style attention (nn_Attention).
#
# reference math (per batch b):
#   h_part = hiddens[b] @ Wd[:DH]                # [S, A]
#   feat   = tanh(h_part + pattern[b] @ Wd[DH:] + bd)
#   score  = feat @ Wv + bv                      # [S, 1]
#   w      = softmax(score over S)               # mask is all-ones
#   out[b] = sum_s w[s] * hiddens[b, s]          # [DH]
#
# Strategy: data-parallel over batch across 8 cores (4 batches/core),
# weights replicated.  Scores are tanh-bounded (|score| <~ 25) so the
# softmax is computed unnormalized: acc = sum exp(s)*h, l = sum exp(s),
# out = acc / l -- a single pass over hiddens, nothing big materialized.
#
# The host stages hiddens pre-transposed per core ([DH, S] per batch) so
# the device reads it exactly once, d-major -- the layout both consumers
# want.  Per-core dataflow (bf16 compute, f32 accumulation):
#   - SWDGE DMA loads hiddensT with f32->bf16 cast: hT [128 d, dj, s]
#   - mm1 (PE): psum[a, s] += Wd_bf[dj, a].T @ hT[dj, s] over 8 d-chunks
#   - ACT: feat = tanh(psum + bias[a]), bias = pattern@Wd_p + bd fused
#     as a per-partition scalar in the [a, s] layout
#   - mm-score (PE): psum[1, s] += Wv[a].T @ feat[a, s] over 4 a-chunks
#   - ACT: e = exp(score + bv) -> [1, S] row; accum_out gives sum(e)
#   - weighted sum on the (otherwise idle) Vector engine:
#     ctx[d] = sum_s hT[d, s] * e[s] via affine_mul_reduce against an
#     e row broadcast across partitions by a tiny ones-matmul -- no
#     transposes needed anywhere on the wide data path
#   - out[b] = ctx / l via a tiny 1/l broadcast matmul + scalar multiply

import numpy as np
from contextlib import ExitStack

B, S, DH, P, A = 32, 2048, 1024, 512, 512
NCORES = 8
BPC = B // NCORES          # batches per core
NT = 4                     # s-tiles of 512 per batch
DCH = DH // 128            # 8 d-chunks
ACH = A // 128             # 4 a-chunks
PCH = P // 128             # 4 p-chunks

_graph_cache = {}


def _force_after(later, earlier):
    # scheduler hint: `later` must come after `earlier` in engine order
    from concourse.tile_rust import add_dep_helper
    li = getattr(later, "instruction", None) or getattr(later, "ins", later)
    ei = getattr(earlier, "instruction", None) or getattr(earlier, "ins", earlier)
    add_dep_helper(li, ei, sync=False, reason="keep tail after chain")


def _build_graph():
    import concourse.bass as bass
    import concourse.mybir as mybir
    import concourse.tile as tile
    from concourse import bacc

    F32 = mybir.dt.float32
    BF16 = mybir.dt.bfloat16
    Act = mybir.ActivationFunctionType

    nc = bacc.Bacc("TRN2", target_bir_lowering=False, debug=False,
                   num_devices=NCORES)

    hT_in = nc.dram_tensor("hiddensT", [BPC, DH, S], F32, kind="ExternalInput").ap()
    wd_in = nc.dram_tensor("Wd", [DH + P, A], F32, kind="ExternalInput").ap()
    # cpack[:, 0:4]=bd, [:, 4:8]=Wv, [:, 8:24]=patternT (c-major), [:, 24]=bv
    cp_in = nc.dram_tensor("cpack", [128, 25], F32, kind="ExternalInput").ap()
    out = nc.dram_tensor("out", [BPC, 128, DCH], F32, kind="ExternalOutput").ap()

    with tile.TileContext(nc) as tc:
        with ExitStack() as es:
            _body(es, tc, nc, mybir, F32, BF16, Act,
                  out, hT_in, wd_in, cp_in)
    # run_bass_via_pjrt binds the exec primitive directly and skips the
    # finalize that runs bacc's register-allocation pass -- do it here.
    nc.finalize()
    return nc


def _body(es, tc, nc, mybir, F32, BF16, Act, out, hT_in, wd_in, cp_in):
    const = es.enter_context(tc.tile_pool(name="const", bufs=1))
    hpool = es.enter_context(tc.tile_pool(name="hp", bufs=4))
    fpool = es.enter_context(tc.tile_pool(name="fp", bufs=2))
    epool = es.enter_context(tc.tile_pool(name="ep", bufs=2))
    opool = es.enter_context(tc.tile_pool(name="op", bufs=4))
    ps_mm1 = es.enter_context(tc.tile_pool(name="ps_mm1", bufs=2, space="PSUM"))
    ps_sc = es.enter_context(tc.tile_pool(name="ps_sc", bufs=2, space="PSUM"))
    ps_ebc = es.enter_context(tc.tile_pool(name="ps_ebc", bufs=4, space="PSUM"))

    # ---- constants / weights ----
    # SWDGE queue: Wd a-cols 0:256 first (so mm1 a=0/1 can start), then
    # batch 0's first small s-slice; the packed small constants ride the
    # HWDGE queue in parallel and are cast/sliced on-chip
    wd_bf = const.tile([128, DCH + PCH, A], BF16, tag="wd")
    wd_src = wd_in.rearrange("(c p) a -> p c a", p=128)
    nc.gpsimd.dma_start(wd_bf[:, :, 0:256], wd_src[:, :, 0:256])

    cpack = const.tile([128, 25], F32, tag="cpack")
    nc.sync.dma_start(cpack[:], cp_in[:])
    bd_sb = cpack[:, 0:4]
    bv_sb = cpack[0:1, 24:25]
    wv_bf = const.tile([128, ACH], BF16, tag="wv")
    nc.scalar.activation(wv_bf[:], cpack[:, 4:8], Act.Identity)
    patT_bf = const.tile([128, PCH * BPC], BF16, tag="patT")
    nc.scalar.activation(patT_bf[:], cpack[:, 8:24], Act.Identity)

    hT0 = hpool.tile([128, DCH, S], BF16, tag="h")
    h0src = hT_in[0].rearrange("(j p) s -> p j s", p=128)
    nc.gpsimd.dma_start(hT0[:, :, 0:256], h0src[:, :, 0:256])
    nc.gpsimd.dma_start(wd_bf[:, :, 256:512], wd_src[:, :, 256:512])
    for sl in [slice(256, 512), slice(512, 1024), slice(1024, 1536),
               slice(1536, 2048)]:
        nc.gpsimd.dma_start(hT0[:, :, sl], h0src[:, :, sl])
    # rows of ones for partition-broadcast matmuls (e rows, 1/l)
    ones_f32 = const.tile([1, 128], F32, tag="ones")
    nc.vector.memset(ones_f32[:], 1.0)
    ones_bf = const.tile([1, 128], BF16, tag="onesb")
    nc.vector.memset(ones_bf[:], 1.0)

    # bias_ab[a, achunk, batch] = (pattern[b] @ Wd_p + bd)[a]; emitted
    # after the first tile's mm1 matmuls (see _emit_bias) so the PE
    # stream is not blocked on the second Wd half at startup
    bias_ab = const.tile([128, ACH, BPC], F32, tag="bias")

    def _emit_bias():
        for a in range(ACH):
            ps_pp = ps_sc.tile([128, 512], F32, tag="sc")
            for k in range(PCH):
                nc.tensor.matmul(
                    ps_pp[:, :BPC],
                    wd_bf[:, DCH + k, a * 128:(a + 1) * 128],
                    patT_bf[:, k * BPC:(k + 1) * BPC],
                    start=(k == 0), stop=(k == PCH - 1),
                )
            nc.vector.tensor_scalar_add(bias_ab[:, a, :], ps_pp[:, :BPC],
                                        bd_sb[:, a:a + 1])


    # ---- main loop over batches ----
    l_rcp_all = epool.tile([1, BPC], F32, tag="lrcpall")
    ctx_list = []
    for b in range(BPC):
        # load hT[b] as bf16: [128 d-part, 8 d-chunk, 2048 s], one DMA per
        # 512-s slice so mm1 of tile t starts as soon as slice t lands
        if b == 0:
            hT = hT0
        else:
            hT = hpool.tile([128, DCH, S], BF16, tag="h")
            hsrc = hT_in[b].rearrange("(j p) s -> p j s", p=128)
            for t in range(NT):
                sl = slice(t * 512, (t + 1) * 512)
                nc.gpsimd.dma_start(hT[:, :, sl], hsrc[:, :, sl])

        e_row = epool.tile([1, S], BF16, tag="erow")
        l_parts = epool.tile([1, NT], F32, tag="lparts")
        e_ps_t = [None] * NT

        for t in range(NT):
            sl = slice(t * 512, (t + 1) * 512)
            # mm1 + tanh -> feat [a-part, achunk, s]
            feat = fpool.tile([128, ACH, 512], BF16, tag="feat")
            first = (b == 0 and t == 0)
            ps1s = [None] * ACH
            for a in range(ACH):
                ps1 = ps_mm1.tile([128, 512], F32, tag="mm1")
                ps1s[a] = ps1
                for dj in range(DCH):
                    nc.tensor.matmul(
                        ps1[:],
                        wd_bf[:, dj, a * 128:(a + 1) * 128],
                        hT[:, dj, sl],
                        start=(dj == 0), stop=(dj == DCH - 1),
                    )
                if not first:
                    nc.scalar.activation(feat[:, a, :], ps1[:], Act.Tanh,
                                         bias=bias_ab[:, a, b:b + 1])
            if first:
                _emit_bias()
                for a in range(ACH):
                    nc.scalar.activation(feat[:, a, :], ps1s[a][:], Act.Tanh,
                                         bias=bias_ab[:, a, b:b + 1])

            # score [1, 512]
            ps_s = ps_sc.tile([1, 512], F32, tag="sc")
            for a in range(ACH):
                nc.tensor.matmul(
                    ps_s[:],
                    wv_bf[:, a:a + 1],
                    feat[:, a, :],
                    start=(a == 0), stop=(a == ACH - 1),
                )

            # e = exp(score + bv) into the batch row; l_t = sum(e)
            nc.scalar.activation(e_row[:, sl], ps_s[:], Act.Exp,
                                 bias=bv_sb[:],
                                 accum_out=l_parts[:, t:t + 1])
            # broadcast e across partitions: ones^T @ e_row -> psum
            e_ps_tile = ps_ebc.tile([128, 512], F32, tag="ebc")
            e_ps_t[t] = e_ps_tile
            nc.tensor.matmul(e_ps_t[t][:], ones_bf[:], e_row[:, sl],
                             start=True, stop=True)

        # weighted sum on DVE: ctx[d-part, dj] = sum_s hT[d, dj, s] * e[s]
        # (in1 streams the broadcast e straight from PSUM); chunked so the
        # chain starts before the last exp -- finer on the last batch to
        # shorten the kernel tail
        nch = NT
        csz = S // nch
        ctx_h = opool.tile([128, DCH, NT], F32, tag="ctxh")
        scratch = fpool.tile([128, S // 2], BF16, tag="scratch")
        e_sb = epool.tile([128, S], BF16, tag="ebc_sb")
        for half in range(nch):
            hs = slice(half * csz, (half + 1) * csz)
            last_cast = nc.vector.tensor_copy(e_sb[:, hs], e_ps_t[half][:])
            for dj in range(DCH):
                nc.vector.affine_mul_reduce(
                    out=scratch[:, :csz],
                    accum_out=ctx_h[:, dj, half:half + 1],
                    in0=hT[:, dj, hs],
                    in1=e_sb[:, hs],
                    scale=1.0,
                    bias=0.0,
                )
        ctx_sb = opool.tile([128, DCH], F32, tag="ctx")
        nc.vector.tensor_add(ctx_h[:, :, 0], ctx_h[:, :, 0], ctx_h[:, :, 1])
        nc.vector.tensor_add(ctx_h[:, :, 2], ctx_h[:, :, 2], ctx_h[:, :, 3])
        add3 = nc.vector.tensor_add(ctx_sb[:], ctx_h[:, :, 0], ctx_h[:, :, 2])

        # l sum + reciprocal; ordering edge keeps these late-waiting DVE
        # ops BEHIND the weighted-sum chain in the in-order DVE stream
        l_sum = epool.tile([1, 1], F32, tag="lsum")
        ladd = nc.vector.reduce_sum(l_sum[:], l_parts[:],
                                    axis=mybir.AxisListType.X)
        _force_after(ladd, last_cast)
        nc.vector.reciprocal(l_rcp_all[:, b:b + 1], l_sum[:])
        ctx_list.append(ctx_sb)

    # ---- division tail: one broadcast matmul, then scale + store ----
    ps_l = ps_sc.tile([128, 512], F32, tag="sc")
    nc.tensor.matmul(ps_l[:, :BPC], ones_f32[:], l_rcp_all[:],
                     start=True, stop=True)
    for b in range(BPC):
        out_sb = opool.tile([128, DCH], F32, tag="osb")
        nc.vector.tensor_scalar_mul(out_sb[:], ctx_list[b][:], ps_l[:, b:b + 1])
        nc.sync.dma_start(out[b], out_sb[:])


def _get_graph():
    if "nc" not in _graph_cache:
        _graph_cache["nc"] = _build_graph()
    return _graph_cache["nc"]


def _make_in_maps(hiddens, pattern, Wd, bd, Wv, bv):
    hiddens = np.asarray(hiddens, dtype=np.float32)
    pattern = np.asarray(pattern, dtype=np.float32)
    Wd = np.asarray(Wd, dtype=np.float32)
    bd = np.asarray(bd, dtype=np.float32)
    Wv = np.asarray(Wv, dtype=np.float32)
    bv = np.asarray(bv, dtype=np.float32)
    in_maps = []
    for c in range(NCORES):
        sl = slice(c * BPC, (c + 1) * BPC)
        cpack = np.zeros((128, 25), dtype=np.float32)
        cpack[:, 0:4] = np.asarray(bd, np.float32).reshape(ACH, 128).T
        cpack[:, 4:8] = np.asarray(Wv, np.float32).reshape(ACH, 128).T
        # patternT[p, c*BPC + b] = pattern[b, c*128 + p]
        patT = np.asarray(pattern[sl], np.float32).T.reshape(PCH, 128, BPC)
        cpack[:, 8:24] = patT.transpose(1, 0, 2).reshape(128, PCH * BPC)
        cpack[:, 24] = np.float32(np.asarray(bv).reshape(-1)[0])
        in_maps.append({
            "hiddensT": np.ascontiguousarray(
                hiddens[sl].transpose(0, 2, 1), dtype=np.float32),
            "Wd": np.ascontiguousarray(Wd, dtype=np.float32),
            "cpack": cpack,
        })
    return in_maps


def run(hiddens, pattern, mask, Wd, bd, Wv, bv, trace=False, **spmd_kwargs):
    from concourse.bass_utils import run_bass_kernel_spmd
    nc = _get_graph()
    in_maps = _make_in_maps(hiddens, pattern, Wd, bd, Wv, bv)
    res = run_bass_kernel_spmd(nc, in_maps, core_ids=list(range(NCORES)),
                               trace=trace, **spmd_kwargs)
    # device emits [BPC, 128, DCH] with d = dj*128 + p; unpermute here
    outs = [np.asarray(res.results[c]["out"]).transpose(0, 2, 1).reshape(BPC, DH)
            for c in range(NCORES)]
    full = np.concatenate(outs, axis=0).astype(np.float32)
    return full, res


def kernel(hiddens, pattern, mask, Wd, bd, Wv, bv):
    full, _ = run(hiddens, pattern, mask, Wd, bd, Wv, bv, trace=False)
    return full
